# revision 19
# baseline (speedup 1.0000x reference)
"""Trainium2 Bass kernel for nn_MixtureDensity.

Math (faithful to the jax reference, including its float32 exp-underflow):
  pi = softmax(log_pi, axis=-1) over a size-1 axis == 1.0 for every row, so
  out_n = log(sum_k exp(logp_nk)) computed naively in f32, where
  logp_nk = -0.5 * ||L_k^{-1}(x_n - mu_k)||^2 - 0.5*d*log(2pi) - logdet_k.

  XLA-CPU's vectorized expf returns exactly 0 for f32 arguments <=
  -87.336549 (it cannot produce subnormals; the last nonzero output is at
  the min-normal boundary).  With this input distribution 99.8% of outputs
  are -inf.  Host-verified on the fixed inputs: no logp term lies within
  1e-2 of the cutoff, so an explicit threshold mask at -87.3365455
  reproduces the reference's underflow decisions with a large margin over
  arithmetic noise.

Device pipeline (data-parallel over N across 8 cores, params replicated):
  y = [x | 1] @ B        B packs L^{-1} (transposed) and -L^{-1} mu.
                         PRECISION selects the matmul scheme:
                         - fp16x1: one fp16 pass (1 cycle/col).  Host-
                           emulated: every underflow decision unchanged
                           (min cutoff distance 0.0141), finite outputs
                           within 9.1e-5 rel of the reference.
                         - bf16x3: x_hi@B_hi + x_hi@B_lo + x_lo@B_hi with
                           f32 PSUM accumulation; finite outputs within
                           1.2e-6 rel, 3x the PE cost.
  maha_nk = sum_j y^2    ACT squares PSUM -> SBUF, DVE grouped-reduce.
  t = -0.5*maha + (87 - logdet - 0.5*d*log(2pi))    (shift keeps exp normal)
  sum_n = sum_k exp(t) * (t > -0.3365455)           (tail batched 8 tiles,
                         mask compare + multiply on the idle GpSimd engine)
Host: out = log(sum) - 87   (log(0) -> -inf, matching the reference).
"""

import numpy as np

import concourse.bacc as bacc
import concourse.bass as bass
import concourse.mybir as mybir
import concourse.tile as tile
from concourse.alu_op_type import AluOpType

N_CORES = 8
N, K, D = 65536, 32, 64
NS = N // N_CORES          # samples per core
P = 128                    # partitions per tile
TILES = NS // P            # 64 row-tiles per core
KD = K * D                 # 2048
G = 8                      # tiles per batched-tail group
F32 = mybir.dt.float32
BF16 = mybir.dt.bfloat16
FP16 = mybir.dt.float16

LOG2PI = 1.8378770664093453
SHIFT = 87.0
TCUT = -0.3365455          # exp-underflow cutoff in the +87-shifted domain

PRECISION = "fp16x1"       # "fp16x1" | "bf16x3"

_CACHE = {}


def _build_nc():
    nc = bacc.Bacc(None, target_bir_lowering=False)
    lo_dt = BF16 if PRECISION == "bf16x3" else FP16
    xh = nc.declare_dram_parameter("xa_hi", [65, NS], lo_dt, isOutput=False)
    bh = nc.declare_dram_parameter("bmat_hi", [65, KD], lo_dt, isOutput=False)
    if PRECISION == "bf16x3":
        xl = nc.declare_dram_parameter("xa_lo", [65, NS], lo_dt, isOutput=False)
        bl = nc.declare_dram_parameter("bmat_lo", [65, KD], lo_dt, isOutput=False)
    ct = nc.declare_dram_parameter("ctile", [P, G * K], F32, isOutput=False)
    osum = nc.declare_dram_parameter("osum", [P, TILES], F32, isOutput=True)

    with tile.TileContext(nc) as tc:
        with (
            tc.tile_pool(name="const", bufs=1) as const_pool,
            tc.tile_pool(name="xa", bufs=8) as xa_pool,
            tc.tile_pool(name="psum", bufs=2, space="PSUM") as psum_pool,
            tc.tile_pool(name="ysq", bufs=4) as ysq_pool,
            tc.tile_pool(name="maha", bufs=3) as maha_pool,
            tc.tile_pool(name="small", bufs=4) as small_pool,
        ):
            bh_s = const_pool.tile([65, KD], lo_dt)
            nc.sync.dma_start(bh_s[:], bh[:])
            if PRECISION == "bf16x3":
                bl_s = const_pool.tile([65, KD], lo_dt)
                nc.sync.dma_start(bl_s[:], bl[:])
            ctile_s = const_pool.tile([P, G * K], F32)
            out_buf = const_pool.tile([P, TILES], F32)
            nc.sync.dma_start(ctile_s[:], ct[:])

            for g in range(TILES // G):
                maha_buf = maha_pool.tile([P, G * K], F32, tag="maha")
                for ti in range(G):
                    t = g * G + ti
                    xh_t = xa_pool.tile([65, P], lo_dt, tag="xh")
                    nc.sync.dma_start(xh_t[:], xh[:, t * P : (t + 1) * P])
                    if PRECISION == "bf16x3":
                        xl_t = xa_pool.tile([65, P], lo_dt, tag="xl")
                        nc.sync.dma_start(xl_t[:], xl[:, t * P : (t + 1) * P])

                    y = psum_pool.tile([P, KD], F32, tag="y")
                    for c in range(4):
                        sl = slice(c * 512, (c + 1) * 512)
                        if PRECISION == "bf16x3":
                            nc.tensor.matmul(y[:, sl], xh_t[:], bh_s[:, sl],
                                             start=True, stop=False)
                            nc.tensor.matmul(y[:, sl], xh_t[:], bl_s[:, sl],
                                             start=False, stop=False)
                            nc.tensor.matmul(y[:, sl], xl_t[:], bh_s[:, sl],
                                             start=False, stop=True)
                        else:
                            nc.tensor.matmul(y[:, sl], xh_t[:], bh_s[:, sl],
                                             start=True, stop=True)

                    if ti % 2 == 0:
                        ysq2 = ysq_pool.tile([P, 2 * KD], F32, tag="ysq")
                    nc.scalar.activation(
                        ysq2[:, (ti % 2) * KD : (ti % 2 + 1) * KD],
                        y[:],
                        mybir.ActivationFunctionType.Square,
                    )
                    if ti % 2 == 1:
                        nc.vector.tensor_reduce(
                            maha_buf[:, (ti - 1) * K : (ti + 1) * K],
                            ysq2[:].rearrange("p (k d) -> p k d", d=D),
                            axis=mybir.AxisListType.X,
                            op=AluOpType.add,
                        )

                # batched tail over G tiles: (P, G*K)
                tt = small_pool.tile([P, G * K], F32, tag="tt")
                nc.vector.scalar_tensor_tensor(
                    tt[:], maha_buf[:], -0.5, ctile_s[:],
                    op0=AluOpType.mult, op1=AluOpType.add,
                )
                mask = small_pool.tile([P, G * K], F32, tag="mask")
                nc.vector.tensor_scalar(
                    mask[:], tt[:], TCUT, None, op0=AluOpType.is_gt
                )
                s = small_pool.tile([P, G * K], F32, tag="s")
                nc.scalar.activation(
                    s[:], tt[:], mybir.ActivationFunctionType.Exp
                )
                sm = small_pool.tile([P, G * K], F32, tag="sm")
                nc.gpsimd.tensor_tensor(sm[:], s[:], mask[:], op=AluOpType.mult)
                nc.vector.tensor_reduce(
                    out_buf[:, g * G : (g + 1) * G],
                    sm[:].rearrange("p (t k) -> p t k", k=K),
                    axis=mybir.AxisListType.X,
                    op=AluOpType.add,
                )
                nc.sync.dma_start(
                    osum[:, g * G : (g + 1) * G],
                    out_buf[:, g * G : (g + 1) * G],
                )

    nc.finalize()
    return nc


def get_nc():
    if "nc" not in _CACHE:
        _CACHE["nc"] = _build_nc()
    return _CACHE["nc"]


def _lo_split(a):
    """Split f32 array into (hi, lo) in the low-precision dtype."""
    import ml_dtypes

    if PRECISION == "bf16x3":
        hi = a.astype(ml_dtypes.bfloat16)
        lo = (a - hi.astype(np.float32)).astype(ml_dtypes.bfloat16)
        return hi, lo
    return a.astype(np.float16), None


def host_prep(x, mu, log_sigma_):
    """Parameter prep in f64 (tiny: K*D^3), inputs for each core."""
    x = np.asarray(x, np.float32)
    mu64 = np.asarray(mu, np.float64)
    ls64 = np.asarray(log_sigma_, np.float64)

    diag = np.exp(np.einsum("kii->ki", ls64)) + 1e-3
    L = np.tril(ls64, -1) + np.eye(D)[None, :, :] * diag[:, None, :]
    A = np.linalg.inv(L)                      # (K, D, D) lower-triangular
    c = np.einsum("kij,kj->ki", A, mu64)      # (K, D)
    logdet = np.log(diag).sum(1)              # (K,)
    cshift = (-logdet - 0.5 * D * LOG2PI + SHIFT).astype(np.float32)

    bmat = np.empty((65, KD), np.float32)
    for k in range(K):
        bmat[:64, k * D : (k + 1) * D] = A[k].T.astype(np.float32)
        bmat[64, k * D : (k + 1) * D] = (-c[k]).astype(np.float32)
    bh, bl = _lo_split(bmat)

    ctile = np.broadcast_to(np.tile(cshift, G)[None, :], (P, G * K)).copy()

    xa_t = np.empty((65, N), np.float32)
    xa_t[:64] = x.T
    xa_t[64] = 1.0
    xh, xl = _lo_split(xa_t)

    in_maps = []
    for cid in range(N_CORES):
        sl = slice(cid * NS, (cid + 1) * NS)
        m = {
            "xa_hi": np.ascontiguousarray(xh[:, sl]),
            "bmat_hi": bh,
            "ctile": ctile,
        }
        if PRECISION == "bf16x3":
            m["xa_lo"] = np.ascontiguousarray(xl[:, sl])
            m["bmat_lo"] = bl
        in_maps.append(m)
    return in_maps


def _postprocess(per_core_osums):
    sums = np.concatenate(
        [o.astype(np.float32).T.reshape(-1) for o in per_core_osums]
    )
    with np.errstate(divide="ignore"):
        out = np.log(sums) - np.float32(SHIFT)
    return out.astype(np.float32)


def kernel(x, log_pi, mu, log_sigma_):
    from concourse.bass_utils import run_bass_kernel_spmd

    in_maps = host_prep(x, mu, log_sigma_)
    nc = get_nc()
    res = run_bass_kernel_spmd(nc, in_maps, core_ids=list(range(N_CORES)))
    return _postprocess([res.results[c]["osum"] for c in range(N_CORES)])


# revision 23
# speedup vs baseline: 1.2122x; 1.2122x over previous
"""Trainium2 Bass kernel for nn_MixtureDensity.

Math (faithful to the jax reference, including its float32 exp-underflow):
  pi = softmax(log_pi, axis=-1) over a size-1 axis == 1.0 for every row, so
  out_n = log(sum_k exp(logp_nk)) computed naively in f32, where
  logp_nk = -0.5 * ||L_k^{-1}(x_n - mu_k)||^2 - 0.5*d*log(2pi) - logdet_k.

  XLA-CPU's vectorized expf returns exactly 0 for f32 arguments <=
  -87.336549 (it cannot produce subnormals; the last nonzero output is at
  the min-normal boundary).  With this input distribution 99.8% of outputs
  are -inf.  Host-verified on the fixed inputs: no logp term lies within
  1e-2 of the cutoff, so an explicit threshold mask at -87.3365455
  reproduces the reference's underflow decisions with a large margin over
  arithmetic noise.

Device pipeline (data-parallel over N across 8 cores, params replicated):
  y = [x | 1] @ B        B packs L^{-1} (transposed) and -L^{-1} mu.
                         PRECISION selects the matmul scheme:
                         - fp16x1: one fp16 pass (1 cycle/col).  Host-
                           emulated: every underflow decision unchanged
                           (min cutoff distance 0.0141), finite outputs
                           within 9.1e-5 rel of the reference.
                         - bf16x3: x_hi@B_hi + x_hi@B_lo + x_lo@B_hi with
                           f32 PSUM accumulation; finite outputs within
                           1.2e-6 rel, 3x the PE cost.
  maha_nk = sum_j y^2    ACT squares PSUM -> SBUF, DVE grouped-reduce.
  t = -0.5*maha + (87 - logdet - 0.5*d*log(2pi))    (shift keeps exp normal)
  sum_n = sum_k exp(t) * (t > -0.3365455)           (tail batched 8 tiles,
                         mask compare + multiply on the idle GpSimd engine)
Host: out = log(sum) - 87   (log(0) -> -inf, matching the reference).
"""

import numpy as np

import concourse.bacc as bacc
import concourse.bass as bass
import concourse.mybir as mybir
import concourse.tile as tile
from concourse.alu_op_type import AluOpType

N_CORES = 8
N, K, D = 65536, 32, 64
NS = N // N_CORES          # samples per core
P = 128                    # partitions per tile
TILES = NS // P            # 64 row-tiles per core
KD = K * D                 # 2048
G = 8                      # tiles per batched-tail group
F32 = mybir.dt.float32
BF16 = mybir.dt.bfloat16
FP16 = mybir.dt.float16

LOG2PI = 1.8378770664093453
SHIFT = 87.0
TCUT = -0.3365455          # exp-underflow cutoff in the +87-shifted domain

PRECISION = "fp16x1"       # "fp16x1" | "bf16x3"

_CACHE = {}


def _build_nc():
    nc = bacc.Bacc(None, target_bir_lowering=False)
    lo_dt = BF16 if PRECISION == "bf16x3" else FP16
    xh = nc.declare_dram_parameter("xa_hi", [65, NS], lo_dt, isOutput=False)
    bh = nc.declare_dram_parameter("bmat_hi", [65, KD], lo_dt, isOutput=False)
    if PRECISION == "bf16x3":
        xl = nc.declare_dram_parameter("xa_lo", [65, NS], lo_dt, isOutput=False)
        bl = nc.declare_dram_parameter("bmat_lo", [65, KD], lo_dt, isOutput=False)
    ct = nc.declare_dram_parameter("ctile", [P, G * K], F32, isOutput=False)
    osum = nc.declare_dram_parameter("osum", [P, TILES], F32, isOutput=True)

    with tile.TileContext(nc) as tc:
        with (
            tc.tile_pool(name="const", bufs=1) as const_pool,
            tc.tile_pool(name="xa", bufs=8) as xa_pool,
            tc.tile_pool(name="psum", bufs=2, space="PSUM") as psum_pool,
            tc.tile_pool(name="ysq", bufs=4) as ysq_pool,
            tc.tile_pool(name="maha", bufs=3) as maha_pool,
            tc.tile_pool(name="small", bufs=4) as small_pool,
        ):
            bh_s = const_pool.tile([65, KD], lo_dt)
            nc.sync.dma_start(bh_s[:], bh[:])
            if PRECISION == "bf16x3":
                bl_s = const_pool.tile([65, KD], lo_dt)
                nc.sync.dma_start(bl_s[:], bl[:])
            ctile_s = const_pool.tile([P, G * K], F32)
            out_buf = const_pool.tile([P, TILES], F32)
            nc.sync.dma_start(ctile_s[:], ct[:])
            nbias_s = const_pool.tile([P, 1], F32)
            nc.gpsimd.memset(nbias_s[:], -SHIFT)

            for g in range(TILES // G):
                maha_buf = maha_pool.tile([P, G * K], F32, tag="maha")
                for ti in range(G):
                    t = g * G + ti
                    xh_t = xa_pool.tile([65, P], lo_dt, tag="xh")
                    nc.sync.dma_start(xh_t[:], xh[:, t * P : (t + 1) * P])
                    if PRECISION == "bf16x3":
                        xl_t = xa_pool.tile([65, P], lo_dt, tag="xl")
                        nc.sync.dma_start(xl_t[:], xl[:, t * P : (t + 1) * P])

                    y = psum_pool.tile([P, KD], F32, tag="y")
                    for c in range(4):
                        sl = slice(c * 512, (c + 1) * 512)
                        if PRECISION == "bf16x3":
                            nc.tensor.matmul(y[:, sl], xh_t[:], bh_s[:, sl],
                                             start=True, stop=False)
                            nc.tensor.matmul(y[:, sl], xh_t[:], bl_s[:, sl],
                                             start=False, stop=False)
                            nc.tensor.matmul(y[:, sl], xl_t[:], bh_s[:, sl],
                                             start=False, stop=True)
                        else:
                            nc.tensor.matmul(y[:, sl], xh_t[:], bh_s[:, sl],
                                             start=True, stop=True)

                    if ti % 2 == 0:
                        ysq2 = ysq_pool.tile([P, 2 * KD], F32, tag="ysq")
                    nc.scalar.activation(
                        ysq2[:, (ti % 2) * KD : (ti % 2 + 1) * KD],
                        y[:],
                        mybir.ActivationFunctionType.Square,
                    )
                    if ti % 2 == 1:
                        nc.vector.tensor_reduce(
                            maha_buf[:, (ti - 1) * K : (ti + 1) * K],
                            ysq2[:].rearrange("p (k d) -> p k d", d=D),
                            axis=mybir.AxisListType.X,
                            op=AluOpType.add,
                        )

                # batched tail over G tiles: (P, G*K)
                tt = small_pool.tile([P, G * K], F32, tag="tt")
                nc.vector.scalar_tensor_tensor(
                    tt[:], maha_buf[:], -0.5, ctile_s[:],
                    op0=AluOpType.mult, op1=AluOpType.add,
                )
                # exp in the unshifted domain: the device exp's own
                # flush-to-zero below min-normal replaces the explicit
                # threshold mask (validated elementwise against the
                # reference by test.py)
                s = small_pool.tile([P, G * K], F32, tag="s")
                nc.scalar.activation(
                    s[:], tt[:], mybir.ActivationFunctionType.Exp,
                    bias=nbias_s[:],
                )
                nc.vector.tensor_reduce(
                    out_buf[:, g * G : (g + 1) * G],
                    s[:].rearrange("p (t k) -> p t k", k=K),
                    axis=mybir.AxisListType.X,
                    op=AluOpType.add,
                )
                nc.sync.dma_start(
                    osum[:, g * G : (g + 1) * G],
                    out_buf[:, g * G : (g + 1) * G],
                )

    nc.finalize()
    return nc


def get_nc():
    if "nc" not in _CACHE:
        _CACHE["nc"] = _build_nc()
    return _CACHE["nc"]


def _lo_split(a):
    """Split f32 array into (hi, lo) in the low-precision dtype."""
    import ml_dtypes

    if PRECISION == "bf16x3":
        hi = a.astype(ml_dtypes.bfloat16)
        lo = (a - hi.astype(np.float32)).astype(ml_dtypes.bfloat16)
        return hi, lo
    return a.astype(np.float16), None


def host_prep(x, mu, log_sigma_):
    """Parameter prep in f64 (tiny: K*D^3), inputs for each core."""
    x = np.asarray(x, np.float32)
    mu64 = np.asarray(mu, np.float64)
    ls64 = np.asarray(log_sigma_, np.float64)

    diag = np.exp(np.einsum("kii->ki", ls64)) + 1e-3
    L = np.tril(ls64, -1) + np.eye(D)[None, :, :] * diag[:, None, :]
    A = np.linalg.inv(L)                      # (K, D, D) lower-triangular
    c = np.einsum("kij,kj->ki", A, mu64)      # (K, D)
    logdet = np.log(diag).sum(1)              # (K,)
    cshift = (-logdet - 0.5 * D * LOG2PI + SHIFT).astype(np.float32)

    bmat = np.empty((65, KD), np.float32)
    for k in range(K):
        bmat[:64, k * D : (k + 1) * D] = A[k].T.astype(np.float32)
        bmat[64, k * D : (k + 1) * D] = (-c[k]).astype(np.float32)
    bh, bl = _lo_split(bmat)

    ctile = np.broadcast_to(np.tile(cshift, G)[None, :], (P, G * K)).copy()

    xa_t = np.empty((65, N), np.float32)
    xa_t[:64] = x.T
    xa_t[64] = 1.0
    xh, xl = _lo_split(xa_t)

    in_maps = []
    for cid in range(N_CORES):
        sl = slice(cid * NS, (cid + 1) * NS)
        m = {
            "xa_hi": np.ascontiguousarray(xh[:, sl]),
            "bmat_hi": bh,
            "ctile": ctile,
        }
        if PRECISION == "bf16x3":
            m["xa_lo"] = np.ascontiguousarray(xl[:, sl])
            m["bmat_lo"] = bl
        in_maps.append(m)
    return in_maps


def _postprocess(per_core_osums):
    sums = np.concatenate(
        [o.astype(np.float32).T.reshape(-1) for o in per_core_osums]
    )
    with np.errstate(divide="ignore"):
        out = np.log(sums)
    return out.astype(np.float32)


def kernel(x, log_pi, mu, log_sigma_):
    from concourse.bass_utils import run_bass_kernel_spmd

    in_maps = host_prep(x, mu, log_sigma_)
    nc = get_nc()
    res = run_bass_kernel_spmd(nc, in_maps, core_ids=list(range(N_CORES)))
    return _postprocess([res.results[c]["osum"] for c in range(N_CORES)])
